# revision 29
# baseline (speedup 1.0000x reference)
"""HOPE block kernel for 8 Trainium2 NeuronCores.

Sequence-parallel sharding: core c owns timesteps [256c, 256(c+1)) of all 4
batches (1024 tokens) and computes ALL 8 heads locally.  The linear-attention
memory M = cumsum_t(mean_b v k^T) is split into a local (within-shard) masked
scan plus a cross-core prefix: each core AllGathers its per-shard memory sum
G_c (8 heads x 64x64, 64KB bf16) and folds sum_{c'<c} G_c' in with a
0/1-mask matmul.  Everything else (LN1/QKV/scan/Wo/LN2/CMS) is local.

ln scales/biases and the 1/B factor are folded into the projection weights
host-side; all weights are pre-arranged host-side so every DMA is a plain
[128, contiguous] transfer.
"""

import numpy as np
import ml_dtypes

import concourse.bass as bass
import concourse.bacc as bacc
import concourse.mybir as mybir
import concourse.tile as tile
from concourse.bass_utils import run_bass_kernel_spmd
from concourse.masks import make_identity

N_CORES = 8
B, S, DIM = 4, 2048, 512
H, D = 8, 64
HID = 4 * DIM
NLVL = 3
EPS = 1e-5
SSH = S // N_CORES       # 256 timesteps per core
TSH = B * SSH            # 1024 tokens per core
NT = TSH // 128          # 8 token tiles (tile t = chunk(t//4), batch(t%4))
NCH = SSH // 128         # 2 chunks of 128 steps
FP32 = mybir.dt.float32
BF16 = mybir.dt.bfloat16
AX = mybir.AxisListType.X
ALU = mybir.AluOpType
ACTF = mybir.ActivationFunctionType


def _ln_normalize(nc, pool, xt, out_bf, sq_scratch, eps_tile, veng=None):
    """out_bf = (xt - mean(xt)) * rsqrt(var(xt)+EPS), per 128-token tile."""
    stats = pool.tile([128, 6], FP32, tag="ln_s")
    mv = pool.tile([128, 2], FP32, tag="ln_m")
    nc.vector.bn_stats(stats[:], xt[:])
    nc.vector.bn_aggr(mv[:], stats[:])
    std = pool.tile([128, 1], FP32, tag="ln_d")
    nc.scalar.activation(std[:], mv[:, 1:2], ACTF.Sqrt, bias=eps_tile[:])
    rs = pool.tile([128, 1], FP32, tag="ln_r")
    nc.vector.reciprocal(rs[:], std[:])
    nc.vector.tensor_scalar(
        out=out_bf[:], in0=xt[:], scalar1=mv[:, 0:1], scalar2=rs[:],
        op0=ALU.subtract, op1=ALU.mult,
    )


def build_kernel():
    nc = bacc.Bacc(num_devices=N_CORES)

    x_t = nc.dram_tensor("x_t", [128, NT, DIM], FP32, kind="ExternalInput")
    qkw = nc.dram_tensor("qkw", [128, 4, H, 128], BF16, kind="ExternalInput")
    qk_b = nc.dram_tensor("qk_b", [128, H], FP32, kind="ExternalInput")
    wkT = nc.dram_tensor("wkT", [128, 4, DIM], BF16, kind="ExternalInput")
    wvT = nc.dram_tensor("wvT", [128, 4, DIM], BF16, kind="ExternalInput")
    bkv = nc.dram_tensor("bkv", [128, 2 * DIM], FP32, kind="ExternalInput")
    woT = nc.dram_tensor("woT", [128, 4, DIM], BF16, kind="ExternalInput")
    sel = nc.dram_tensor("sel", [128, 4, D], BF16, kind="ExternalInput")
    w1 = nc.dram_tensor("w1", [NLVL, 128, 4 * 16 * 128], BF16, kind="ExternalInput")
    w2 = nc.dram_tensor("w2", [NLVL, 128, 16 * 4 * 128], BF16, kind="ExternalInput")
    b1 = nc.dram_tensor("b1", [NLVL, 128, HID // 128], FP32, kind="ExternalInput")
    b2a = nc.dram_tensor("b2a", [2, 128, DIM // 128], FP32, kind="ExternalInput")
    b2last = nc.dram_tensor("b2last", [128, DIM], FP32, kind="ExternalInput")
    out_t = nc.dram_tensor("out_t", [128, NT, DIM], FP32, kind="ExternalOutput")

    with tile.TileContext(nc) as tc:
        with tc.tile_pool(name="dram", bufs=1, space="DRAM") as dram, \
             tc.tile_pool(name="const", bufs=1) as cpool, \
             tc.tile_pool(name="lns", bufs=4) as lnp, \
             tc.tile_pool(name="xp", bufs=1) as xpool:

            ag_in = dram.tile([D, DIM], BF16)
            ag_out = dram.tile([N_CORES * D, DIM], BF16, addr_space="Shared")

            # x first: everything downstream waits on it (split across
            # DMA queues)
            x_sb = xpool.tile([128, NT, DIM], FP32)
            for xh in range(4):
                nc.sync.dma_start(x_sb[:, 2 * xh:2 * xh + 2], x_t[:, 2 * xh:2 * xh + 2])

            identity = cpool.tile([128, 128], BF16)
            make_identity(nc, identity[:])
            # keep-mask tiled over 4 batch column blocks: mask[t, s%128]=1 if t<=s
            mask = cpool.tile([128, 512], FP32)
            nc.gpsimd.memset(mask[:], 1.0)
            for bb in range(4):
                nc.gpsimd.affine_select(
                    out=mask[:, bb * 128:(bb + 1) * 128],
                    in_=mask[:, bb * 128:(bb + 1) * 128],
                    compare_op=ALU.is_ge, fill=0.0,
                    base=0, pattern=[[1, 128]], channel_multiplier=-1,
                )

            qkw_sb = cpool.tile([128, 4, H, 128], BF16)
            nc.sync.dma_start(qkw_sb[:], qkw[:])
            qkb_sb = cpool.tile([128, H], FP32)
            nc.sync.dma_start(qkb_sb[:], qk_b[:])
            wkT_sb = cpool.tile([128, 4, DIM], BF16)
            nc.sync.dma_start(wkT_sb[:], wkT[:])
            wvT_sb = cpool.tile([128, 4, DIM], BF16)
            nc.sync.dma_start(wvT_sb[:], wvT[:])
            bkv_sb = cpool.tile([128, 2 * DIM], FP32)
            nc.sync.dma_start(bkv_sb[:], bkv[:])
            woT_sb = cpool.tile([128, 4, DIM], BF16)
            nc.sync.dma_start(woT_sb[:], woT[:])
            sel_sb = cpool.tile([128, 4, D], BF16)
            nc.sync.dma_start(sel_sb[:], sel[:])
            b2l_sb = cpool.tile([128, DIM], FP32)
            nc.sync.dma_start(b2l_sb[:], b2last[:])
            eps_sb = cpool.tile([128, 1], FP32)
            nc.vector.memset(eps_sb[:], EPS)

            # long-lived attention tiles
            yT_p = tc.tile_pool(name="yTp", bufs=1)
            yTpool = yT_p.__enter__()
            yT = yTpool.tile([128, 4, TSH], BF16)
            yTc = yTpool.tile([128, 4, TSH], BF16, name="yTc")

            with tc.tile_pool(name="kv", bufs=1) as kvp, \
                 tc.tile_pool(name="qk", bufs=1) as qkp, \
                 tc.tile_pool(name="s1w", bufs=3) as s1w:
                QT = qkp.tile([128, H // 2, TSH], BF16)
                KT = qkp.tile([128, H // 2, TSH], BF16)
                K_td = kvp.tile([128, NT, DIM], BF16)
                V_td = kvp.tile([128, NT, DIM], BF16)
                GcA_sb = kvp.tile([D, DIM], FP32)
                GcB_sb = kvp.tile([D, DIM], FP32)
                Gc_bf = kvp.tile([D, DIM], BF16)

                hT_ctx = tc.tile_pool(name="hT", bufs=1)
                hTp = hT_ctx.__enter__()
                hT = hTp.tile([128, 4, TSH], BF16)
                # ---- stage 1+2a fused per tile: ln1 -> transpose -> K/V
                # projection, so the PE starts projecting as soon as the first
                # tile is normalized ----
                with tc.tile_pool(name="s1p", bufs=2, space="PSUM") as s1p, \
                     tc.tile_pool(name="s2aw", bufs=2, space="PSUM") as s2ap, \
                     tc.tile_pool(name="s2g", bufs=1, space="PSUM") as s2gp:
                    pgA = s2gp.tile([D, DIM], FP32)
                    pgB = s2gp.tile([D, DIM], FP32)
                    for t in range(NT):
                        tcol = slice(t * 128, (t + 1) * 128)
                        hn = s1w.tile([128, DIM], BF16, tag="hn")
                        sq = s1w.tile([128, DIM], BF16, tag="sq")
                        _ln_normalize(nc, lnp, x_sb[:, t], hn, sq, eps_sb)
                        for a in range(4):
                            ps = s1p.tile([128, 128], BF16)
                            nc.tensor.transpose(ps[:], hn[:, a * 128:(a + 1) * 128],
                                                identity[:])
                            nc.vector.tensor_copy(hT[:, a, tcol], ps[:])
                        psK = s2ap.tile([128, DIM], FP32, tag="psK")
                        psV = s2ap.tile([128, DIM], FP32, tag="psV")
                        for a in range(4):
                            nc.tensor.matmul(psK[:], hT[:, a, tcol], wkT_sb[:, a],
                                             start=(a == 0), stop=(a == 3))
                        for a in range(4):
                            nc.tensor.matmul(psV[:], hT[:, a, tcol], wvT_sb[:, a],
                                             start=(a == 0), stop=(a == 3))
                        nc.vector.tensor_tensor(K_td[:, t], psK[:], bkv_sb[:, 0:DIM],
                                                ALU.add)
                        nc.vector.tensor_tensor(V_td[:, t], psV[:], bkv_sb[:, DIM:],
                                                ALU.add)
                    # local memory sums; each accumulation group must be
                    # contiguous matmul instructions
                    for pg, t0 in ((pgA, 0), (pgB, 4)):
                        for h in range(H):
                            hc = slice(h * D, (h + 1) * D)
                            for tt in range(4):
                                nc.tensor.matmul(pg[:, hc], K_td[:, t0 + tt, hc],
                                                 V_td[:, t0 + tt, hc],
                                                 start=(tt == 0), stop=(tt == 3))
                    nc.vector.tensor_copy(GcA_sb[:], pgA[:])
                    nc.vector.tensor_copy(GcB_sb[:], pgB[:])
                    nc.vector.tensor_tensor(Gc_bf[:], GcA_sb[:], GcB_sb[:], ALU.add)
                    nc.sync.dma_start(ag_in[:], Gc_bf[:])

                nc.gpsimd.collective_compute(
                    "AllGather", ALU.bypass,
                    replica_groups=[list(range(N_CORES))],
                    ins=[ag_in.opt()], outs=[ag_out.opt()],
                )

                # agg DMA fires as soon as the collective lands
                agg_sb = kvp.tile([128, 4, DIM], BF16, name="agg_sb")
                nc.sync.dma_start(
                    agg_sb[:], ag_out[:].rearrange("(a p) m -> p a m", p=128))

                # ---- stage 2b: Q/K pair-block projections, software-pipelined
                # with the causal T = mask o (K^T Q) matmuls AND the local
                # (prefix-free) readout y_local = sum_bp V_bp Tm_bp, so the PE
                # runs one continuous stream while the collective flies.  Heads
                # are stored in pairs (even head on partitions 0-63, odd on
                # 64-127) so paired T (K=64) and y (M=64) matmuls land on
                # distinct PE row/col-groups and execute concurrently.  The
                # cross-core prefix term G @ Q is added later, folded into the
                # Wo matmul's accumulation group via yTc. ----
                tm_ctx = tc.tile_pool(name="tm", bufs=64)
                tmp_pool = tm_ctx.__enter__()
                tms = {}
                with tc.tile_pool(name="s2bp", bufs=2, space="PSUM") as s2bp, \
                     tc.tile_pool(name="pt3", bufs=4, space="PSUM") as pt3, \
                     tc.tile_pool(name="py3", bufs=1, space="PSUM") as py3:
                    def emit_qk(hp):
                        for qk in range(2):
                            blk = 2 * hp + qk
                            dst = QT if qk == 0 else KT
                            for nh in range(2):
                                ncol = slice(nh * 512, (nh + 1) * 512)
                                pqk = s2bp.tile([128, 512], FP32)
                                for a in range(4):
                                    nc.tensor.matmul(pqk[:], qkw_sb[:, a, blk],
                                                     hT[:, a, ncol],
                                                     start=(a == 0), stop=(a == 3))
                                nc.scalar.activation(dst[:, hp, ncol], pqk[:],
                                                     ACTF.Identity,
                                                     bias=qkb_sb[:, blk:blk + 1])

                    def emit_T(hp):
                        for sc in range(NCH):
                            qcol = slice(sc * 512, (sc + 1) * 512)
                            for bp in range(B):
                                kcol = slice((sc * 4 + bp) * 128,
                                             (sc * 4 + bp) * 128 + 128)
                                for e in range(2):
                                    rows = slice(e * D, (e + 1) * D)
                                    pt = pt3.tile([128, 512], FP32)
                                    nc.tensor.matmul(pt[:], KT[rows, hp, kcol],
                                                     QT[rows, hp, qcol])
                                    tm = tmp_pool.tile([128, 512], BF16)
                                    nc.vector.tensor_tensor(tm[:], pt[:], mask[:],
                                                            ALU.mult)
                                    tms[(sc, 2 * hp + e, bp)] = tm

                    def emit_ylocal(hp):
                        for sc in range(NCH):
                            qcol = slice(sc * 512, (sc + 1) * 512)
                            py_e = py3.tile([128, 512], FP32, tag="pye")
                            py_o = py3.tile([128, 512], FP32, tag="pyo")
                            for e, py in ((0, py_e), (1, py_o)):
                                h = 2 * hp + e
                                hc = slice(h * D, (h + 1) * D)
                                out = py[e * D:(e + 1) * D, :]
                                for bp in range(B):
                                    nc.tensor.matmul(out, V_td[:, sc * 4 + bp, hc],
                                                     tms[(sc, h, bp)][:],
                                                     start=(bp == 0),
                                                     stop=(bp == B - 1))
                            nc.scalar.activation(yT[0:D, hp, qcol], py_e[0:D, :],
                                                 ACTF.Copy)
                            nc.scalar.activation(yT[D:128, hp, qcol], py_o[D:128, :],
                                                 ACTF.Copy)

                    for hp in range(H // 2 + 2):
                        if hp < H // 2:
                            emit_qk(hp)
                        if 1 <= hp <= H // 2:
                            emit_T(hp - 1)
                        if hp >= 2:
                            emit_ylocal(hp - 2)
                tm_ctx.__exit__(None, None, None)
                hT_ctx.__exit__(None, None, None)

                # ---- stage 3: cross-core prefix fold -> yTc ----
                with tc.tile_pool(name="gt", bufs=1) as gtp, \
                     tc.tile_pool(name="pyc", bufs=2, space="PSUM") as pycp, \
                     tc.tile_pool(name="psh", bufs=2, space="PSUM") as pshp, \
                     tc.tile_pool(name="pgp", bufs=1, space="PSUM") as pgpp:
                    pgp = pgpp.tile([D, DIM], FP32)
                    for a in range(4):
                        nc.tensor.matmul(pgp[:], sel_sb[:, a], agg_sb[:, a],
                                         start=(a == 0), stop=(a == 3))
                    # G tiles duplicated onto partitions 64-127 (via an
                    # identity shift-matmul) so odd-head corrections can pair
                    G0_bf = gtp.tile([128, DIM], BF16, name="G0_bf")
                    G1_bf = gtp.tile([128, DIM], BF16, name="G1_bf")
                    nc.vector.tensor_copy(G0_bf[0:D, :], pgp[:])
                    nc.vector.tensor_tensor(G1_bf[0:D, :], pgp[:], GcA_sb[:], ALU.add)
                    for Gt in (G0_bf, G1_bf):
                        psh = pshp.tile([128, DIM], FP32)
                        nc.tensor.matmul(psh[D:128, :], identity[0:D, 0:D],
                                         Gt[0:D, :])
                        nc.vector.tensor_copy(Gt[D:128, :], psh[D:128, :])
                    for sc in range(NCH):
                        qcol = slice(sc * 512, (sc + 1) * 512)
                        Gsc = G0_bf if sc == 0 else G1_bf
                        for hp in range(H // 2):
                            pyc_e = pycp.tile([128, 512], FP32, tag="pce")
                            pyc_o = pycp.tile([128, 512], FP32, tag="pco")
                            for e, pyc in ((0, pyc_e), (1, pyc_o)):
                                h = 2 * hp + e
                                hc = slice(h * D, (h + 1) * D)
                                rows = slice(e * D, (e + 1) * D)
                                nc.tensor.matmul(pyc[rows, :], Gsc[rows, hc],
                                                 QT[rows, hp, qcol])
                            nc.scalar.activation(yTc[0:D, hp, qcol], pyc_e[0:D, :],
                                                 ACTF.Copy)
                            nc.scalar.activation(yTc[D:128, hp, qcol],
                                                 pyc_o[D:128, :], ACTF.Copy)

            # CMS weight pool + level-0 prefetch (SBUF freed by attention)
            wts_ctx = tc.tile_pool(name="cmsw", bufs=2)
            wts = wts_ctx.__enter__()
            w1_sb0 = wts.tile([128, 4, 16, 128], BF16, tag="w1")
            nc.sync.dma_start(w1_sb0[:], w1[0].rearrange("p (a h q) -> p a h q", a=4, h=16))
            w2_sb0 = wts.tile([128, 16, 4, 128], BF16, tag="w2")
            nc.sync.dma_start(w2_sb0[:], w2[0].rearrange("p (h a q) -> p h a q", h=16, a=4))

            # ---- stage 4: Wo + residual + ln2 + transpose ----
            h2nT_p = tc.tile_pool(name="h2nT", bufs=1)
            h2_p = tc.tile_pool(name="h2", bufs=1)
            h2nT = h2nT_p.__enter__().tile([128, 4, TSH], BF16)
            h2_sb = h2_p.__enter__().tile([128, NT, DIM], FP32)
            with tc.tile_pool(name="s4w", bufs=4) as s4w, \
                 tc.tile_pool(name="s4p", bufs=2, space="PSUM") as s4p, \
                 tc.tile_pool(name="s4pt", bufs=2, space="PSUM") as s4pt:
                for t in range(NT):
                    tcol = slice(t * 128, (t + 1) * 128)
                    po = s4p.tile([128, DIM], FP32)
                    for a in range(4):
                        nc.tensor.matmul(po[:], yT[:, a, tcol], woT_sb[:, a],
                                         start=(a == 0), stop=False)
                    for a in range(4):
                        nc.tensor.matmul(po[:], yTc[:, a, tcol], woT_sb[:, a],
                                         start=False, stop=(a == 3))
                    nc.vector.tensor_tensor(h2_sb[:, t], po[:], x_sb[:, t], ALU.add)
                    hn = s4w.tile([128, DIM], BF16, tag="hn2")
                    sq = s4w.tile([128, DIM], BF16, tag="sq2")
                    _ln_normalize(nc, lnp, h2_sb[:, t], hn, sq, eps_sb)
                    for a in range(4):
                        ps = s4pt.tile([128, 128], BF16)
                        nc.tensor.transpose(ps[:], hn[:, a * 128:(a + 1) * 128], identity[:])
                        nc.vector.tensor_copy(h2nT[:, a, t * 128:(t + 1) * 128], ps[:])

            # ---- stage 5: CMS chain ----
            with tc.tile_pool(name="g", bufs=1) as gp, \
                 tc.tile_pool(name="bts", bufs=2) as bts, \
                 tc.tile_pool(name="s5o", bufs=2) as s5o, \
                 tc.tile_pool(name="s5p", bufs=8, space="PSUM") as s5p:
                g_sb = gp.tile([128, 16, TSH], BF16)
                cur = h2nT
                for lvl in range(NLVL):
                    if lvl == 0:
                        w1_sb = w1_sb0
                    else:
                        w1_sb = wts.tile([128, 4, 16, 128], BF16, tag="w1")
                        nc.sync.dma_start(
                            w1_sb[:],
                            w1[lvl].rearrange("p (a h q) -> p a h q", a=4, h=16))
                    b1_sb = bts.tile([128, HID // 128], FP32, tag="b1")
                    nc.sync.dma_start(b1_sb[:], b1[lvl])
                    for ht in range(16):
                        for nh in range(2):
                            colw = slice(nh * 512, nh * 512 + 512)
                            ps = s5p.tile([128, 512], FP32)
                            for a in range(4):
                                nc.tensor.matmul(ps[:], w1_sb[:, a, ht],
                                                 cur[:, a, colw],
                                                 start=(a == 0), stop=(a == 3))
                            nc.scalar.activation(
                                g_sb[:, ht, colw], ps[:], ACTF.Gelu_apprx_tanh,
                                bias=b1_sb[:, ht:ht + 1])
                    if lvl == 0:
                        w2_sb = w2_sb0
                    else:
                        w2_sb = wts.tile([128, 16, 4, 128], BF16, tag="w2")
                        nc.sync.dma_start(
                            w2_sb[:],
                            w2[lvl].rearrange("p (h a q) -> p h a q", h=16, a=4))
                    if lvl < 2:
                        b2_sb = bts.tile([128, 4], FP32, tag="b2")
                        nc.sync.dma_start(b2_sb[:], b2a[lvl])
                        nxt = s5o.tile([128, 4, TSH], BF16, tag="nxt")
                        for a in range(4):
                            for nh in range(2):
                                colw = slice(nh * 512, nh * 512 + 512)
                                ps = s5p.tile([128, 512], FP32)
                                for ht in range(16):
                                    nc.tensor.matmul(ps[:], w2_sb[:, ht, a],
                                                     g_sb[:, ht, colw],
                                                     start=(ht == 0), stop=(ht == 15))
                                nc.scalar.activation(
                                    nxt[:, a, colw], ps[:], ACTF.Identity,
                                    bias=b2_sb[:, a:a + 1])
                        cur = nxt
                    else:
                        # last level emits [tok, dim]; add b2 + residual, write out
                        w2r = w2_sb[:].rearrange("p h a q -> p h (a q)")
                        for t in range(NT):
                            ps = s5p.tile([128, 512], FP32)
                            for ht in range(16):
                                nc.tensor.matmul(
                                    ps[:], g_sb[:, ht, t * 128:(t + 1) * 128],
                                    w2r[:, ht],
                                    start=(ht == 0), stop=(ht == 15))
                            tmp = s5o.tile([128, DIM], FP32, tag="fin")
                            nc.vector.tensor_tensor(tmp[:], ps[:], b2l_sb[:], ALU.add)
                            nc.vector.tensor_tensor(tmp[:], tmp[:], h2_sb[:, t], ALU.add)
                            nc.sync.dma_start(out_t[:, t], tmp[:])
            h2_p.__exit__(None, None, None)
            h2nT_p.__exit__(None, None, None)
            wts_ctx.__exit__(None, None, None)
            yT_p.__exit__(None, None, None)

    nc.finalize()
    return nc


_NC_CACHE = {}


def _get_nc():
    if "nc" not in _NC_CACHE:
        _NC_CACHE["nc"] = build_kernel()
    return _NC_CACHE["nc"]


def kernel(x, Wq, Wk, Wv, Wo, ln1_w, ln1_b, ln2_w, ln2_b,
           cms_W1, cms_b1, cms_W2, cms_b2):
    bf = ml_dtypes.bfloat16
    f32 = np.float32
    x = np.asarray(x, f32)
    ln1_w = np.asarray(ln1_w, f32); ln1_b = np.asarray(ln1_b, f32)
    ln2_w = np.asarray(ln2_w, f32); ln2_b = np.asarray(ln2_b, f32)

    Wq = np.asarray(Wq, f32); Wk = np.asarray(Wk, f32); Wv = np.asarray(Wv, f32)
    Wo = np.asarray(Wo, f32)
    Wqs = Wq * ln1_w[None, :]; Wks = Wk * ln1_w[None, :]
    Wvs = (Wv * ln1_w[None, :]) / B
    bq = Wq @ ln1_b; bk = Wk @ ln1_b; bv = (Wv @ ln1_b) / B

    W1 = np.asarray(cms_W1, f32).copy(); b1v = np.asarray(cms_b1, f32).copy()
    W2 = np.asarray(cms_W2, f32); b2v = np.asarray(cms_b2, f32)
    b1v[0] = b1v[0] + ln2_b @ W1[0]
    W1[0] = W1[0] * ln2_w[:, None]

    # [128, 4a, 8blk, 128m]: block 2hp = q of head pair hp (even head on
    # rows 0-63, odd on 64-127), block 2hp+1 = k of the same pair
    Wq_h = Wqs.reshape(H, D, DIM)
    Wk_h = Wks.reshape(H, D, DIM)
    blocks, bcols = [], []
    for hp in range(H // 2):
        blocks.append(np.concatenate([Wq_h[2 * hp], Wq_h[2 * hp + 1]], axis=0))
        blocks.append(np.concatenate([Wk_h[2 * hp], Wk_h[2 * hp + 1]], axis=0))
        bcols.append(np.concatenate([bq[(2 * hp) * D:(2 * hp + 1) * D],
                                     bq[(2 * hp + 1) * D:(2 * hp + 2) * D]]))
        bcols.append(np.concatenate([bk[(2 * hp) * D:(2 * hp + 1) * D],
                                     bk[(2 * hp + 1) * D:(2 * hp + 2) * D]]))
    QK = np.stack(blocks, axis=0)                      # [8, 128, DIM]
    qkw_arr = QK.transpose(2, 0, 1).reshape(4, 128, H, 128).transpose(1, 0, 2, 3)
    qkb_arr = np.stack(bcols, axis=1)                  # [128, 8]
    wkT_arr = Wks.T.reshape(4, 128, DIM).transpose(1, 0, 2)
    wvT_arr = Wvs.T.reshape(4, 128, DIM).transpose(1, 0, 2)
    bkv_arr = np.broadcast_to(np.concatenate([bk, bv]), (128, 2 * DIM))
    woT_arr = Wo.T.reshape(4, 128, DIM).transpose(1, 0, 2)

    w1_arr = W1.reshape(NLVL, 4, 128, 16, 128).transpose(0, 2, 1, 3, 4).reshape(
        NLVL, 128, 4 * 16 * 128)
    w2_arr = W2.reshape(NLVL, 16, 128, 4, 128).transpose(0, 2, 1, 3, 4).reshape(
        NLVL, 128, 16 * 4 * 128)
    b1r = np.ascontiguousarray(b1v.reshape(NLVL, HID // 128, 128).transpose(0, 2, 1))
    b2ar = np.ascontiguousarray(b2v[:2].reshape(2, DIM // 128, 128).transpose(0, 2, 1))
    b2lr = np.broadcast_to(b2v[2], (128, DIM)).copy()

    rows = np.arange(512)
    in_maps = []
    for c in range(N_CORES):
        xs = x[:, c * SSH:(c + 1) * SSH, :]            # [4, 256, 512]
        x_tiled = xs.reshape(B, NCH, 128, DIM).transpose(2, 1, 0, 3).reshape(
            128, NT, DIM)
        sel_arr = ((rows // D < c)[:, None] &
                   ((rows % D)[:, None] == np.arange(D)[None, :])).astype(f32)
        sel_t = sel_arr.reshape(4, 128, D).transpose(1, 0, 2)
        in_maps.append({
            "x_t": np.ascontiguousarray(x_tiled),
            "qkw": qkw_arr.astype(bf),
            "qk_b": qkb_arr.astype(f32),
            "wkT": wkT_arr.astype(bf),
            "wvT": wvT_arr.astype(bf),
            "bkv": bkv_arr.astype(f32),
            "woT": woT_arr.astype(bf),
            "sel": sel_t.astype(bf),
            "w1": w1_arr.astype(bf),
            "w2": w2_arr.astype(bf),
            "b1": b1r.astype(f32),
            "b2a": b2ar.astype(f32),
            "b2last": b2lr.astype(f32),
        })

    nc = _get_nc()
    res = run_bass_kernel_spmd(nc, in_maps, core_ids=list(range(N_CORES)))
    _NC_CACHE["last_result"] = res
    out = np.empty((B, S, DIM), dtype=f32)
    for c in range(N_CORES):
        r = res.results[c]["out_t"]                    # [128, 8, 512]
        out[:, c * SSH:(c + 1) * SSH, :] = r.reshape(
            128, NCH, B, DIM).transpose(2, 1, 0, 3).reshape(B, SSH, DIM)
    return out
